# revision 27
# baseline (speedup 1.0000x reference)
"""AttentionDCA energy kernel for 8 Trainium2 NeuronCores (fp8 DoubleRow).

Math: with one-hot E_b in {0,1}^{L x 21} for sequence x[b],
    energy[b] = -sum_h <E_b^T A_h E_b, V_h>_F
where A_h = softmax(Q_h K_h^T / d_k).  Everything becomes PE matmuls;
phase 2 runs fp8e4m3 DoubleRow (halved instruction count), the softmax
numerators and one-hots travel as fp8, the normalized row-blocks as bf16.

  per core (4 heads, H sharded over 8 cores):
    S_T[j,i]   = K_h Q_h^T                  (fp8 scores, transposed layout)
    ex         = exp(S_T / d_k)             (ScalarE, PSUM->SBUF fp8)
    R'[i,col]  = sum_j ex[j,i] * EA[j,col]  (PE DoubleRow; EA = one-hot for all
                                             64 b + a 1/16-column -> r/16)
    r_hat      = R' * (16/r[i])             (ScalarE evicts the ones-chunk,
                                             VectorE the two 504-col chunks,
                                             both into a contiguous h-major
                                             bf16 row block)
    C          = E_g^T r_hat_g              (PE; stationary = a 128-col window
                                             of EA itself -> FWL, no separate
                                             E2 copy)
    S[p,g]     = sum((C * -1/16) * V)       (one fused DVE scalar_tensor_tensor
                                             with free-axis accumulate)
  host: 21-row segment sums of the [128, 11] per-core S, summed over cores.

v2 notes vs v1 (traced):
  - input DMAs are chained (qk0 -> {eb, qk1} -> vv) so qk0 gets full HBM
    bandwidth and the first score matmul starts ~1us earlier.
  - phase-2 ones-chunk (379 cols incl the 1/16 column) accumulates FIRST in
    its own 1-bank psum tile, so reciprocal + ScalarE eviction overlap the
    remaining 2-bank DoubleRow matmuls instead of gating the next iteration.
  - r_hat rows are h-major and each eviction is contiguous (no 4-D rearrange
    APs on the DVE).
  - phase 3 loads its stationary from eb directly (group g = cols
    126g..126g+128; the 2 overhang cols only feed vv rows 126/127 which are
    zero) and streams r_hat via a strided [4h x 126] AP.
  - phase 4 (selector matmul + copy) is gone: the [128, 11] accumulator DMAs
    out and the 21-row sums happen on host with the cross-core reduce.
"""

import numpy as np
import ml_dtypes

# Problem constants (hardcoded per contract)
B, L, H, D, NAA = 64, 512, 32, 128, 21
NCORES = 8
HPC = H // NCORES            # heads per core = 4
JB = L // 128                # 4 position blocks
BG = 6                       # b's per group
NG = (B + BG - 1) // BG      # 11 groups
BPAD = BG * NG               # 66 padded batch
NDATA = BPAD * NAA           # 1386 one-hot columns (64 real b's + 2 zero pad)
ONES_COL = NDATA             # 1386: 1/16-column -> r/16 row sums
NEA = 1392                   # eb width: 1386 data + ones col + 5 zero pad
MP = BG * NAA                # 126 output partitions used per group
GW = MP                      # 126: group stride in eb's column space
NQ = HPC * MP                # 504: phase-3 free size (h-major)
CW = 504                     # phase-2 chunk width for the two full banks
CW2 = NDATA - 2 * CW         # 378 data cols in the ones chunk
RW = 1392                    # per-(ib,h) r_hat row width (1386 + pad)
NWARM = 9                   # PE warm-up matmuls bridging the qk0 DMA window

_NC = None


def _build_nc():
    import concourse.bacc as bacc
    import concourse.tile as tile
    from concourse import mybir

    f32 = mybir.dt.float32
    bf16 = mybir.dt.bfloat16
    fp8 = mybir.dt.float8e4
    AF = mybir.ActivationFunctionType
    DR = mybir.MatmulPerfMode.DoubleRow
    mult = mybir.AluOpType.mult

    nc = bacc.Bacc("TRN2", target_bir_lowering=False, debug=False)

    qk_d = [
        nc.dram_tensor(f"qk{h}", [128, 2, L], fp8, kind="ExternalInput")
        for h in range(HPC)
    ]
    eb_d = nc.dram_tensor("eb", [128, JB, NEA], fp8, kind="ExternalInput")
    vv_d = nc.dram_tensor("vv", [128, NQ], bf16, kind="ExternalInput")
    out_d = nc.dram_tensor("energy", [128, NG], f32, kind="ExternalOutput")

    with tile.TileContext(nc) as tc:
        with (
            tc.tile_pool(name="const", bufs=1) as cpool,
            tc.tile_pool(name="exps", bufs=HPC) as xpool,
            tc.tile_pool(name="rall", bufs=1) as rpool,
            tc.tile_pool(name="small", bufs=8) as spool,
            tc.tile_pool(name="ps", bufs=2, space="PSUM") as ps_pool,
            tc.tile_pool(name="pr1", bufs=2, space="PSUM") as pr1_pool,
            tc.tile_pool(name="pr2", bufs=2, space="PSUM") as pr2_pool,
        ):
            qk_sb = [
                cpool.tile([128, 2, L], fp8, tag=f"qk{h}", name=f"qk{h}_sb")
                for h in range(HPC)
            ]
            eb_sb = cpool.tile([128, JB, NEA], fp8, tag="eb")
            vv_sb = cpool.tile([128, NQ], bf16, tag="vv")
            s_sb = cpool.tile([128, NG], f32, tag="ssb")
            wz_l = cpool.tile([128, 128], fp8, tag="wzl")
            wz_r = cpool.tile([128, 256], fp8, tag="wzr")

            # All input DMAs run concurrently (chaining costs ~2us per link:
            # completion-sem visibility + ~1.1us queue ring-start latency).
            # qk[0] (gates the first scores) and eb (gates the first mat1)
            # are each split across the two HWDGE engines (SP + ScalarE) so
            # each gets two queues' worth of bandwidth and finishes in
            # need-order; the per-head qk split lets scores(0) start on head
            # 0's 128KB instead of the full 512KB.  ScalarE only issues two
            # so its exp-table load isn't delayed.
            nc.sync.dma_start(out=qk_sb[0][0:64], in_=qk_d[0][0:64])
            nc.scalar.dma_start(out=qk_sb[0][64:128], in_=qk_d[0][64:128])
            nc.sync.dma_start(out=eb_sb[0:32], in_=eb_d[0:32])
            nc.scalar.dma_start(out=eb_sb[64:128], in_=eb_d[64:128])
            nc.sync.dma_start(out=eb_sb[32:64], in_=eb_d[32:64])
            nc.sync.dma_start(out=qk_sb[1][:], in_=qk_d[1][:])
            nc.sync.dma_start(out=qk_sb[2][:], in_=qk_d[2][:])
            nc.sync.dma_start(out=qk_sb[3][:], in_=qk_d[3][:])
            nc.sync.dma_start(out=vv_sb[:], in_=vv_d[:])
            nc.vector.memset(wz_l[:], 0.0)
            nc.vector.memset(wz_r[:], 0.0)

            # PE warm-up: matmuls on zeroed tiles with no DMA dependency keep
            # the PE busy through the qk0-DMA window so HAM un-throttles
            # before real work.  All warm-ups accumulate into ONE psum tile
            # as a single accumulation group: rotating fresh tiles would
            # stall the PE ~600ns per tile on the release chain, and those
            # gaps keep the HAM busy-window from ever filling.
            pw = ps_pool.tile([128, 512], f32, tag="ps")
            for w in range(NWARM):
                nc.tensor.matmul(pw[:, 0:256], wz_l[:], wz_r[:],
                                 start=(w == 0), stop=(w == NWARM - 1))

            # Phase 1+2 interleaved: scores+exp for head h are emitted two
            # heads ahead of mat1(h), so PE streams score matmuls for h+2
            # while ScalarE runs exp(h+1) and PE's mat1(h) chews on exp(h).
            exps = []

            def scores(h):
                ex = xpool.tile([128, JB, L], fp8, tag="ex")
                exps.append(ex)
                qq = qk_sb[h]
                if h == 0:
                    # startup head: fully pipelined MM/exp alternation so the
                    # exp chain (which gates mat1) starts ASAP; borrow the
                    # (still idle) pr2 psum banks for jb 2/3 so the 4-matmul
                    # chain isn't gated on exp recycling the 2 ps bufs
                    for jb in range(JB):
                        if jb >= 2:
                            pst = pr2_pool.tile([128, 2, 512], f32, tag="pr2")
                            psc = pst[:, jb - 2, :]
                        else:
                            pst = ps_pool.tile([128, 512], f32, tag="ps")
                            psc = pst[:]
                        nc.tensor.matmul(
                            psc,
                            qq[:, 1, jb * 128:(jb + 1) * 128],
                            qq[:, 0, :],
                            start=True,
                            stop=True,
                        )
                        nc.scalar.activation(
                            ex[:, jb, :], psc, AF.Exp, scale=1.0 / D,
                        )
                    return
                # later heads interleave with DoubleRow mat1 matmuls: emit
                # score MMs in adjacent pairs so the PE pays the FWL<->DR
                # weight-path mode switch once per pair instead of per MM
                for base in (0, 2):
                    pss = []
                    for jb in (base, base + 1):
                        pst = ps_pool.tile([128, 512], f32, tag="ps")
                        nc.tensor.matmul(
                            pst[:],
                            qq[:, 1, jb * 128:(jb + 1) * 128],
                            qq[:, 0, :],
                            start=True,
                            stop=True,
                        )
                        pss.append(pst)
                    for k, jb in enumerate((base, base + 1)):
                        nc.scalar.activation(
                            ex[:, jb, :], pss[k][:], AF.Exp, scale=1.0 / D,
                        )

            scores(0)
            scores(1)
            # small filler matmuls bridge the PE-idle window between the
            # last score and the first mat1 (gated on exp(h0) + the eb DMA);
            # an idle gap there re-throttles the HAM clock for all of phase 2
            pf = pr1_pool.tile([128, 512], f32, tag="pr1")
            for w in range(4):
                nc.tensor.matmul(pf[:, 0:256], wz_l[:], wz_r[:],
                                 start=(w == 0), stop=(w == 3))
            # r_hat layout: [p, ib, h, col] with each head's 1386 columns
            # contiguous, so every eviction is a contiguous store
            r_sb = rpool.tile([128, JB, HPC, RW], bf16, tag="r")

            for h in range(HPC):
                ex = exps[h]
                for ib in range(JB):
                    lhs0 = ex[:, 0:2, ib * 128:(ib + 1) * 128]
                    lhs1 = ex[:, 2:4, ib * 128:(ib + 1) * 128]
                    steps = ((0, lhs0), (1, lhs1))
                    # ones-chunk first, in its own 1-bank tile: reciprocal +
                    # its eviction run under the remaining 4 matmuls.  For
                    # h == 3 no scores interleave, so the ps pool is idle
                    # and alternating pa between pr1/ps doubles release slack
                    if h == 3 and ib % 2 == 1:
                        pat = ps_pool.tile([128, 512], f32, tag="ps")
                    else:
                        pat = pr1_pool.tile([128, 512], f32, tag="pr1")
                    pa = pat
                    for s, lhs in steps:
                        nc.tensor.matmul(
                            pa[:, 0:CW2 + 1],
                            lhs,
                            eb_sb[:, 2 * s:2 * s + 2, 2 * CW:2 * CW + CW2 + 1],
                            start=(s == 0),
                            stop=(s == 1),
                            perf_mode=DR,
                        )
                    rcp16 = spool.tile([128, 1], f32, tag="rcp")
                    # ones column holds 1/16 -> accumulated r/16 -> 16/r here
                    nc.vector.reciprocal(rcp16[:], pa[:, CW2:CW2 + 1])
                    pb = pr2_pool.tile([128, 2, 512], f32, tag="pr2")
                    for ck in (0, 1):
                        for s, lhs in steps:
                            nc.tensor.matmul(
                                pb[:, ck, 0:CW],
                                lhs,
                                eb_sb[:, 2 * s:2 * s + 2, ck * CW:(ck + 1) * CW],
                                start=(s == 0),
                                stop=(s == 1),
                                perf_mode=DR,
                            )
                    # evictions (PSUM f32 -> SBUF bf16, scale 16/r).  During
                    # h < 3 ScalarE is busy with the interleaved exps, so the
                    # pa chain (recip -> ones-chunk evict) stays on VectorE
                    # and ScalarE takes one full bank; for h >= 2 ScalarE is
                    # free and the cheaper v8 split (ScalarE: ones chunk,
                    # VectorE: both full banks in one op) balances better.
                    if h < 3:
                        nc.vector.tensor_scalar_mul(
                            r_sb[:, ib, h, 2 * CW:2 * CW + CW2],
                            pa[:, 0:CW2],
                            rcp16[:],
                        )
                        nc.vector.tensor_scalar_mul(
                            r_sb[:, ib, h, 0:CW],
                            pb[:, 0, 0:CW],
                            rcp16[:],
                        )
                        nc.scalar.mul(
                            r_sb[:, ib, h, CW:2 * CW],
                            pb[:, 1, 0:CW],
                            rcp16[:],
                        )
                    else:
                        nc.scalar.mul(
                            r_sb[:, ib, h, 2 * CW:2 * CW + CW2],
                            pa[:, 0:CW2],
                            rcp16[:],
                        )
                        nc.vector.tensor_scalar_mul(
                            r_sb[:, ib, h, 0:2 * CW].rearrange(
                                "p (c w) -> p c w", c=2),
                            pb[:, :, 0:CW],
                            rcp16[:],
                        )
                if h + 2 < HPC:
                    scores(h + 2)

            # Phase 3: C = E_g^T r_hat (bf16 moving via a [4h x 126] strided
            # AP, stationary = 128-col window of eb), then one fused DVE
            # multiply-by-V with free-axis accumulate per group
            pool_cycle = (ps_pool, pr1_pool, pr2_pool)
            for g in range(NG):
                pool = pool_cycle[g % 3]
                if pool is pr2_pool:
                    pct = pool.tile([128, 2, 512], f32, tag="pr2")
                    pc = pct[:, 0, 0:NQ]
                elif pool is pr1_pool:
                    pct = pool.tile([128, 512], f32, tag="pr1")
                    pc = pct[:, 0:NQ]
                else:
                    pct = pool.tile([128, 512], f32, tag="ps")
                    pc = pct[:, 0:NQ]
                for ib in range(JB):
                    nc.tensor.matmul(
                        pc,
                        eb_sb[:, ib, GW * g:GW * g + 128],
                        r_sb[:, ib, :, GW * g:GW * g + GW],
                        start=(ib == 0),
                        stop=(ib == JB - 1),
                    )
                scr = spool.tile([128, NQ], bf16, tag="scr")
                nc.vector.scalar_tensor_tensor(
                    out=scr[:],
                    in0=pc,
                    scalar=-1.0 / 16.0,
                    in1=vv_sb[:],
                    op0=mult,
                    op1=mult,
                    accum_out=s_sb[:, g:g + 1],
                )

            nc.sync.dma_start(out=out_d[:], in_=s_sb[:])

    nc.compile()
    return nc


def _get_nc():
    global _NC
    if _NC is None:
        _NC = _build_nc()
    return _NC


def _stage_inputs(x, Q, K, V):
    """Host-side sharding/staging. Returns in_maps for the 8 cores."""
    fp8 = ml_dtypes.float8_e4m3
    bf16 = ml_dtypes.bfloat16
    x = np.asarray(x)
    Q = np.asarray(Q, dtype=np.float32)
    K = np.asarray(K, dtype=np.float32)
    V = np.asarray(V, dtype=np.float32)

    # One-hot EA [L, NEA] (+ 1/16 column at 1386), replicated to all cores
    onehot = (x[:, :, None] == np.arange(NAA, dtype=x.dtype)[None, None, :])
    ea = np.zeros((L, NEA), dtype=np.float32)
    ea[:, : B * NAA] = onehot.transpose(1, 0, 2).reshape(L, B * NAA)
    ea[:, ONES_COL] = 1.0 / 16.0
    eb_host = np.ascontiguousarray(
        ea.reshape(JB, 128, NEA).transpose(1, 0, 2)
    ).astype(fp8)

    in_maps = []
    for c in range(NCORES):
        hs = slice(c * HPC, (c + 1) * HPC)
        qt = Q[hs].transpose(2, 0, 1)
        kt = K[hs].transpose(2, 0, 1)
        in_map = {"eb": eb_host}
        for h in range(HPC):
            in_map[f"qk{h}"] = np.ascontiguousarray(
                np.stack([qt[:, h], kt[:, h]], axis=1)).astype(fp8)
        vv = np.zeros((128, NQ), dtype=np.float32)
        vc = V[hs]
        for h in range(HPC):
            for bl in range(BG):
                vv[bl * NAA:(bl + 1) * NAA,
                   h * MP + bl * NAA: h * MP + (bl + 1) * NAA] = vc[h]
        in_map["vv"] = vv.astype(bf16)
        in_maps.append(in_map)
    return in_maps


def _reduce_energy(arr):
    """[128, NG] per-core accumulator -> [BG, NG] via 21-row segment sums."""
    return arr[:MP].reshape(BG, NAA, NG).sum(axis=1)


def _run(x, Q, K, V, trace=False):
    from concourse.bass_utils import run_bass_kernel_spmd

    nc = _get_nc()
    in_maps = _stage_inputs(x, Q, K, V)
    res = run_bass_kernel_spmd(nc, in_maps, list(range(NCORES)), trace=trace)

    total = np.zeros((BG, NG), dtype=np.float64)
    for r in res.results:
        total += _reduce_energy(r["energy"].astype(np.float64))
    bidx = np.arange(B)
    energy = total[bidx % BG, bidx // BG].astype(np.float32)
    return energy, res


def kernel(x, Q, K, V):
    return _run(x, Q, K, V)[0]
